# revision 6
# baseline (speedup 1.0000x reference)
"""MoE output combine kernel for Trainium2 (Bass/Tile), 8-core SPMD.

Problem: out[b,s,e] = sum_n routing_logits[b,s,n] * expert_outputs[b,n,s,e]
  B=8, S=4096, N=8, E=128, fp32.

Sharding: batch across the 8 NeuronCores (fully local combine, no
collectives). Each core reads its (4096,8) logits slice and (8,4096,128)
expert slice (~16 MiB) and writes a (4096,128) output (~2 MiB) —
memory-bound at ~19 MB per core.

Per-core layout: s is tiled into blocks of 1024; an s-block of one expert
is a contiguous 512 KB region, loaded as an SBUF tile (128, 1024) where
partition p holds 8 consecutive s-rows (s = s0 + 8p + q, q in [0,8)).
The matching logits block is (128, 64): partition p holds the 64
contiguous floats of logits rows [s0+8p, s0+8p+8).

Compute: for each (q, n) the (128,128) segment gets a fused
multiply-accumulate `acc = x*w + acc` via scalar_tensor_tensor with the
per-partition weight column w[:, q*8+n]. Work is split across engines:
experts 0,6,7 multiplied on the scalar engine (activation Copy with
per-partition scale; 6,7 into scratch tiles), experts 1..5 fused-MAC on
the vector engine, and the two scratch tiles accumulated on gpsimd with
plain tensor_tensor adds (scalar_tensor_tensor is not legal on Pool) —
each engine stays under the ~50us DMA roofline.
"""

import numpy as np

B, S, N, E = 8, 4096, 8, 128
SBLK = 1024          # s-values per block (512 KB per expert per block)
NBLK = S // SBLK     # 4 blocks
QF = SBLK // 128     # 8 q-segments of 128 s-rows per block
P = 128

# experts 1..VEC_SPLIT on vector engine, VEC_SPLIT+1..7 on gpsimd
VEC_SPLIT = 5

_nc_cache = None


def _build_nc():
    import concourse.bacc as bacc
    import concourse.mybir as mybir
    from concourse.tile import TileContext

    f32 = mybir.dt.float32
    mult = mybir.AluOpType.mult
    add = mybir.AluOpType.add
    Copy = mybir.ActivationFunctionType.Copy

    nc = bacc.Bacc("TRN2", target_bir_lowering=False)
    logits = nc.dram_tensor("routing_logits", [S, N], f32, kind="ExternalInput")
    expert = nc.dram_tensor("expert_outputs", [N, S, E], f32, kind="ExternalInput")
    out = nc.dram_tensor("out", [S, E], f32, kind="ExternalOutput")

    with TileContext(nc) as tc:
        with (
            tc.tile_pool(name="xp", bufs=2 * N) as xp,
            tc.tile_pool(name="wp", bufs=NBLK) as wp,
            tc.tile_pool(name="op", bufs=3) as op,
            tc.tile_pool(name="tp", bufs=8) as tp,
        ):
            for blk in range(NBLK):
                s0 = blk * SBLK
                w = wp.tile([P, QF * N], f32, tag="w")
                nc.sync.dma_start(
                    out=w[:],
                    in_=logits[s0 : s0 + SBLK, :].rearrange("(p q) n -> p (q n)", p=P),
                )
                xs = []
                for n in range(N):
                    x = xp.tile([P, SBLK], f32, tag="x")
                    nc.sync.dma_start(
                        out=x[:],
                        in_=expert[n, s0 : s0 + SBLK, :].rearrange(
                            "(p q) e -> p (q e)", p=P
                        ),
                    )
                    xs.append(x)
                o = op.tile([P, SBLK], f32, tag="o")
                for q in range(QF):
                    seg = o[:, q * E : (q + 1) * E]
                    tmps = []
                    for n in range(VEC_SPLIT + 1, N):
                        xseg = xs[n][:, q * E : (q + 1) * E]
                        wcol = w[:, q * N + n : q * N + n + 1]
                        t = tp.tile([P, E], f32, tag="tmp")
                        nc.scalar.activation(t[:], xseg, Copy, scale=wcol)
                        tmps.append(t)
                    for n in range(VEC_SPLIT + 1):
                        xseg = xs[n][:, q * E : (q + 1) * E]
                        wcol = w[:, q * N + n : q * N + n + 1]
                        if n == 0:
                            nc.scalar.activation(seg, xseg, Copy, scale=wcol)
                        else:
                            nc.vector.scalar_tensor_tensor(
                                seg, xseg, wcol, seg, op0=mult, op1=add
                            )
                    for t in tmps:
                        nc.gpsimd.tensor_tensor(
                            out=seg, in0=seg, in1=t[:], op=add
                        )
                nc.sync.dma_start(
                    out=out[s0 : s0 + SBLK, :].rearrange("(p q) e -> p (q e)", p=P),
                    in_=o[:],
                )
    nc.compile()
    return nc


def _get_nc():
    global _nc_cache
    if _nc_cache is None:
        _nc_cache = _build_nc()
    return _nc_cache


def kernel(routing_logits, expert_outputs, _trace=False):
    from concourse.bass_utils import run_bass_kernel_spmd

    logits = np.asarray(routing_logits, dtype=np.float32)
    expert = np.asarray(expert_outputs, dtype=np.float32)
    assert logits.shape == (B, S, N), logits.shape
    assert expert.shape == (B, N, S, E), expert.shape

    nc = _get_nc()
    in_maps = [
        {
            "routing_logits": np.ascontiguousarray(logits[b]),
            "expert_outputs": np.ascontiguousarray(expert[b]),
        }
        for b in range(B)
    ]
    res = run_bass_kernel_spmd(nc, in_maps, core_ids=list(range(B)), trace=_trace)
    out = np.stack([np.asarray(res.results[b]["out"]) for b in range(B)], axis=0)
    if _trace:
        return out, res
    return out
